# revision 12
# baseline (speedup 1.0000x reference)
"""Fused 7-gate continuous-time LSTM cell on 8 Trainium2 NeuronCores.

Data-parallel over batch: each core gets B/8 = 1024 rows; the fused gate
weight W [2048, 7*2048] is replicated. All heavy data is pre-packed and
pre-quantized on the host so the device does zero transposes and every
DMA is wide and contiguous:

  g = hx @ W + b  with per-gate precision:
    - gates i1, i2, o  (bounded multipliers): fp8e4m3 DoubleRow matmuls
      (2 fp8 MACs/cell/cycle) when FP8_GATES is on, else fp16
    - gates f1, f2, z, d (error-sensitive: multiply cx / enter exp):
      fp16 matmuls
  Gate columns are packed in order [i1,i2,o | f1,f2,z,d] so each
  precision group is contiguous.

Per (h-block, m-tile) the PE accumulates the 3 "A" gates into 3 PSUM
banks and the 4 "B" gates into 4 more; DVE drains each half with a
fused (psum*scale + bias) -> fp16, so the two halves ping-pong inside
8 banks with no PE stall. Activations batch per h-block in two ACT
table phases ({sigmoid,tanh} then {exp,ln}); tanh(c_t) is deferred one
h-block so the ACT engine only swaps tables twice per h-block. Cell
math runs on Pool (SBUF-only) + DVE in fp16.
"""

import sys

sys.path.insert(0, "/opt/trn_rl_repo")

import ml_dtypes
import numpy as np

import concourse.bass as bass
import concourse.mybir as mybir
import concourse.tile as tile
from concourse import bacc, bass_utils

B, D, H, NG = 8192, 2048, 2048, 7
N_CORES = 8
BL = B // N_CORES  # 1024 rows per core
P = 128
MT = BL // P  # 8 m-tiles
HBW = 256  # h-block width
NHB = H // HBW  # 8 h-blocks
KT = D // P  # 16 fp16 k-tiles
KS = D // (2 * P)  # 8 fp8 DoubleRow k-supertiles

FP8_GATES = True  # i1,i2,o via fp8e4m3 DoubleRow
S8 = 16.0 * 64.0  # hx*16, W*64 pre-scales for the fp8 gates

# pack order: [i1, i2, o, f1, f2, z, d] (original gate indices)
PACK = [0, 1, 4, 2, 3, 5, 6]
NA, NB = 3, 4  # half A = packed gates 0:3, half B = 3:7

F32 = mybir.dt.float32
F16 = mybir.dt.float16
F8 = mybir.dt.float8e4
AF = mybir.ActivationFunctionType
ALU = mybir.AluOpType
DR = mybir.MatmulPerfMode.DoubleRow

_cached_nc = None


def _build():
    nc = bacc.Bacc("TRN2", target_bir_lowering=False, debug=False,
                   num_devices=N_CORES)
    hxT16 = nc.dram_tensor("hxT16", [P, KT, BL], F16, kind="ExternalInput").ap()
    if FP8_GATES:
        hxT8 = nc.dram_tensor("hxT8", [P, KT, BL], F8, kind="ExternalInput").ap()
        w16 = nc.dram_tensor("w16", [KT, NHB, P, NB, HBW], F16,
                             kind="ExternalInput").ap()
        w8 = nc.dram_tensor("w8", [KS, NHB, P, 2, NA, HBW], F8,
                            kind="ExternalInput").ap()
    else:
        w16 = nc.dram_tensor("w16", [KT, NHB, P, NG, HBW], F16,
                             kind="ExternalInput").ap()
    cx1 = nc.dram_tensor("cx1", [BL, H], F16, kind="ExternalInput").ap()
    cx2 = nc.dram_tensor("cx2", [BL, H], F16, kind="ExternalInput").ap()
    dt_in = nc.dram_tensor("dt", [BL, 1], F32, kind="ExternalInput").ap()
    bias = nc.dram_tensor("bias", [NHB, NG, HBW], F16, kind="ExternalInput").ap()
    out = nc.dram_tensor("out", [3, BL, H], F32, kind="ExternalOutput").ap()

    from contextlib import ExitStack

    with tile.TileContext(nc) as tc, ExitStack() as ctx:
        const_pool = ctx.enter_context(tc.tile_pool(name="const", bufs=1))
        wpool = ctx.enter_context(tc.tile_pool(name="w", bufs=20))
        w8pool = ctx.enter_context(tc.tile_pool(name="w8", bufs=10))
        bpool = ctx.enter_context(tc.tile_pool(name="bb", bufs=3))
        psa_pool = ctx.enter_context(tc.tile_pool(name="psa", bufs=1, space="PSUM"))
        psb_pool = ctx.enter_context(tc.tile_pool(name="psb", bufs=1, space="PSUM"))
        gadd_pool = ctx.enter_context(tc.tile_pool(name="gadd", bufs=11))
        keep_pool = ctx.enter_context(tc.tile_pool(name="keep", bufs=2))
        cx_pool = ctx.enter_context(tc.tile_pool(name="cx", bufs=8))
        out_pool = ctx.enter_context(tc.tile_pool(name="outp", bufs=4))
        tmp_pool = ctx.enter_context(tc.tile_pool(name="tmp", bufs=4))

        def prefetch_w(hb, q):
            w8ts = []
            if FP8_GATES:
                for s in range(KS):
                    w8t = w8pool.tile([P, 2, NA, HBW], F8, tag="w8",
                                      name=f"w8t_{hb}_{s}")
                    q.dma_start(w8t, w8[s, hb])
                    w8ts.append(w8t)
            wts = []
            for k in range(KT):
                wt = wpool.tile([P, NB if FP8_GATES else NG, HBW], F16,
                                tag="w", name=f"wt_{hb}_{k}")
                q.dma_start(wt, w16[k, hb])
                wts.append(wt)
            return w8ts, wts

        # hb0 weights issue on the (startup-idle) gpsimd queue, in parallel
        # with the hx loads on sync; fp8 tensors first (fp8 half runs first)
        wcache = {0: prefetch_w(0, nc.gpsimd)}
        if FP8_GATES:
            hxt8 = const_pool.tile([P, KT, BL], F8)
            for kc in range(0, KT, 4):
                nc.sync.dma_start(hxt8[:, kc : kc + 4, :],
                                  hxT8[:, kc : kc + 4, :])
        hxt = const_pool.tile([P, KT, BL], F16)
        for kc in range(0, KT, 4):
            nc.sync.dma_start(hxt[:, kc : kc + 4, :], hxT16[:, kc : kc + 4, :])

        # -u per batch row, u = dt; laid out [128, m]
        dtt = const_pool.tile([P, MT], F32)
        dt_sq = bass.AP(tensor=dt_in.tensor, offset=0, ap=[[1, P], [P, MT]])
        nc.sync.dma_start(dtt, dt_sq)
        negu = const_pool.tile([P, MT], F32)
        nc.vector.tensor_scalar_mul(negu, dtt, -1.0)

        o_prev = ct_prev = None  # deferred from previous h-block

        for hb in range(NHB):
            # --- weights for this h-block (resident across all m) ---
            w8ts, wts = wcache.pop(hb) if hb in wcache else prefetch_w(hb, nc.sync)

            bsl = bias[hb]  # [NG, HBW]
            b_bcast = bass.AP(
                tensor=bsl.tensor, offset=bsl.offset, ap=[[0, P], *bsl.ap]
            )
            bt = bpool.tile([P, NG, HBW], F16, tag="bt")
            nc.sync.dma_start(bt, b_bcast)

            gadds = []
            cxts = []
            # --- GEMM blocks: one per m-tile ---
            for m in range(MT):
                ms = slice(m * P, (m + 1) * P)
                cs = slice(hb * HBW, (hb + 1) * HBW)
                cx1t = cx_pool.tile([P, HBW], F16, tag="cx1", name=f"cx1_{hb}_{m}")
                nc.sync.dma_start(cx1t, cx1[ms, cs])
                cx2t = cx_pool.tile([P, HBW], F16, tag="cx2", name=f"cx2_{hb}_{m}")
                nc.sync.dma_start(cx2t, cx2[ms, cs])
                cxts.append((cx1t, cx2t))

                psa = psa_pool.tile([P, NA, 2 * HBW], F32, tag="psa",
                                    name=f"psa_{hb}_{m}")
                psb = psb_pool.tile([P, NB, 2 * HBW], F32, tag="psb",
                                    name=f"psb_{hb}_{m}")
                # half A: packed gates 0..2 (i1, i2, o)
                if FP8_GATES:
                    for s in range(KS):
                        lhs8 = hxt8[:, 2 * s : 2 * s + 2, ms]
                        for g in range(NA):
                            nc.tensor.matmul(
                                psa[:, g, 0:HBW],
                                lhs8,
                                w8ts[s][:, :, g, :],
                                start=(s == 0),
                                stop=(s == KS - 1),
                                perf_mode=DR,
                            )
                else:
                    for k in range(KT):
                        lhs = hxt[:, k, ms]
                        for g in range(NA):
                            nc.tensor.matmul(
                                psa[:, g, 0:HBW],
                                lhs,
                                wts[k][:, g, :],
                                start=(k == 0),
                                stop=(k == KT - 1),
                            )
                # half B: packed gates 3..6 (f1, f2, z, d)
                for k in range(KT):
                    lhs = hxt[:, k, ms]
                    for g in range(NB):
                        wcol = g if FP8_GATES else NA + g
                        nc.tensor.matmul(
                            psb[:, g, 0:HBW],
                            lhs,
                            wts[k][:, wcol, :],
                            start=(k == 0),
                            stop=(k == KT - 1),
                        )

                # drains: fused (psum*scale + bias) -> fp16, on DVE
                gadd = gadd_pool.tile([P, NG, HBW], F16, tag="gadd",
                                      name=f"gadd_{hb}_{m}")
                gadds.append(gadd)
                nc.vector.scalar_tensor_tensor(
                    gadd[:, 0:NA, :],
                    psa[:, :, 0:HBW],
                    (1.0 / S8) if FP8_GATES else 1.0,
                    bt[:, 0:NA, :],
                    ALU.mult,
                    ALU.add,
                )
                nc.vector.scalar_tensor_tensor(
                    gadd[:, NA:NG, :],
                    psb[:, :, 0:HBW],
                    1.0,
                    bt[:, NA:NG, :],
                    ALU.mult,
                    ALU.add,
                )

            o_cur = keep_pool.tile([P, MT, HBW], F16, tag="okeep")
            ct_cur = keep_pool.tile([P, MT, HBW], F16, tag="ctkeep")
            cs = slice(hb * HBW, (hb + 1) * HBW)
            last = hb == NHB - 1
            # Last h-block: split phases into two m-batches so most of its
            # post-GEMM work overlaps the final matmul blocks and only
            # m=6..7 remain after the last MM. Cell math sprints on the
            # (then-idle) DVE; elsewhere it runs on Pool.
            eng = nc.vector if last else nc.gpsimd
            m_batches = [range(0, 6), range(6, MT)] if last else [range(MT)]
            for bi, mb in enumerate(m_batches):
                mb = list(mb)
                # --- ACT phase A: {sigmoid, tanh} table ---
                fj = tmp_pool.tile([P, 1], F32, tag="fj")
                for m in mb:
                    g = gadds[m]
                    # i1, i2 in place; o and z to keep-tiles (outlive gadd)
                    nc.scalar.activation(g[:, 0:2, :], g[:, 0:2, :], AF.Sigmoid)
                    nc.scalar.activation(o_cur[:, m, :], g[:, 2, :], AF.Sigmoid)
                    nc.scalar.activation(g[:, 3:5, :], g[:, 3:5, :], AF.Sigmoid)
                    nc.scalar.activation(g[:, 5, :], g[:, 5, :], AF.Tanh,
                                         accum_out=fj if m == mb[-1] else None)
                # fence: phase-B ops read scale/bias tiles derived from the
                # last phase-A op's accum_out, so the ACT queue can't
                # interleave {exp,ln} into the {sigmoid,tanh} batch.
                onesf = tmp_pool.tile([P, 1], F32, tag="onesf")
                nc.vector.tensor_scalar(onesf, fj, 0.0, 1.0, ALU.mult, ALU.add)
                zerof = tmp_pool.tile([P, 1], F32, tag="zerof")
                nc.vector.tensor_scalar_mul(zerof, fj, 0.0)
                # deferred from previous h-block: h_t = o * tanh(c_t)
                # (tanh lives in both ACT tables, so never forces a load)
                if bi == 0 and ct_prev is not None:
                    pcs = slice((hb - 1) * HBW, hb * HBW)
                    for m in range(MT):
                        ms = slice(m * P, (m + 1) * P)
                        tct = tmp_pool.tile([P, HBW], F16, tag="tct")
                        nc.scalar.activation(tct, ct_prev[:, m, :], AF.Tanh)
                        ht = out_pool.tile([P, HBW], F32, tag="ht")
                        nc.gpsimd.tensor_mul(ht, o_prev[:, m, :], tct)
                        nc.sync.dma_start(out[2, ms, pcs], ht)

                # --- ACT phase B: {exp, ln} table; batched per function so
                # each table loads once per batch ---
                exs = {}
                for m in mb:
                    ex = tmp_pool.tile([P, HBW], F16, tag="ex", bufs=MT)
                    nc.scalar.activation(ex, gadds[m][:, 6, :], AF.Exp,
                                         scale=onesf)
                    exs[m] = ex
                for m in mb:
                    nc.scalar.activation(gadds[m][:, 6, :], exs[m], AF.Ln,
                                         bias=1.0, scale=onesf)
                for m in mb:
                    # E = exp(-u * decay)
                    nc.scalar.activation(gadds[m][:, 6, :], gadds[m][:, 6, :],
                                         AF.Exp, scale=negu[:, m : m + 1],
                                         bias=zerof)

                # --- cell math (fp16) ---
                for m in mb:
                    ms = slice(m * P, (m + 1) * P)
                    g = gadds[m]
                    cx1t, cx2t = cxts[m]
                    t1 = tmp_pool.tile([P, HBW], F16, tag="t1")
                    eng.tensor_mul(t1, g[:, 3, :], cx1t)  # f1*cx1
                    t2 = tmp_pool.tile([P, HBW], F16, tag="t2")
                    eng.tensor_mul(t2, g[:, 0, :], g[:, 5, :])  # i1*z
                    cy1 = out_pool.tile([P, HBW], F32, tag="cy1")
                    eng.tensor_add(cy1, t1, t2)
                    nc.sync.dma_start(out[0, ms, cs], cy1)

                    t3 = tmp_pool.tile([P, HBW], F16, tag="t3")
                    eng.tensor_mul(t3, g[:, 4, :], cx2t)  # f2*cx2
                    t4 = tmp_pool.tile([P, HBW], F16, tag="t4")
                    eng.tensor_mul(t4, g[:, 1, :], g[:, 5, :])  # i2*z
                    cy2 = out_pool.tile([P, HBW], F32, tag="cy2")
                    eng.tensor_add(cy2, t3, t4)
                    nc.sync.dma_start(out[1, ms, cs], cy2)

                    dif = tmp_pool.tile([P, HBW], F16, tag="dif")
                    eng.tensor_sub(dif, cy1, cy2)
                    t5 = tmp_pool.tile([P, HBW], F16, tag="t5")
                    eng.tensor_mul(t5, dif, g[:, 6, :])  # (cy1-cy2)*E
                    eng.tensor_add(ct_cur[:, m, :], cy2, t5)
                    if last:
                        # no next phase A to defer into — finish h_t inline
                        # (tanh is in the loaded {exp,ln}-adjacent table)
                        tct = tmp_pool.tile([P, HBW], F16, tag="tct")
                        nc.scalar.activation(tct, ct_cur[:, m, :], AF.Tanh)
                        ht = out_pool.tile([P, HBW], F32, tag="ht")
                        eng.tensor_mul(ht, o_cur[:, m, :], tct)
                        nc.sync.dma_start(out[2, ms, cs], ht)

            o_prev, ct_prev = o_cur, ct_cur

    nc.compile()
    return nc


def _get_nc():
    global _cached_nc
    if _cached_nc is None:
        _cached_nc = _build()
    return _cached_nc


def _pack_host(hx, cx1, cx2, dt, W, b):
    f16 = np.float16
    E4 = ml_dtypes.float8_e4m3
    Wr = np.ascontiguousarray(
        W.reshape(D, NG, H)[:, PACK, :], dtype=np.float32
    )  # [k, g', h] in packed gate order
    if FP8_GATES:
        w16 = np.ascontiguousarray(
            Wr[:, NA:, :].astype(f16)
            .reshape(KT, P, NB, NHB, HBW)
            .transpose(0, 3, 1, 2, 4)
        )  # [KT, NHB, P, NB, HBW]
        w8 = np.ascontiguousarray(
            (Wr[:, :NA, :] * 64.0).astype(E4)
            .reshape(KS, 2, P, NA, NHB, HBW)
            .transpose(0, 4, 2, 1, 3, 5)
        )  # [KS, NHB, P, 2, NA, HBW]
    else:
        w16 = np.ascontiguousarray(
            Wr.astype(f16).reshape(KT, P, NG, NHB, HBW).transpose(0, 3, 1, 2, 4)
        )
        w8 = None
    bp = np.ascontiguousarray(
        b.reshape(NG, H)[PACK].reshape(NG, NHB, HBW).transpose(1, 0, 2)
    ).astype(f16)  # [NHB, NG, HBW]
    cx1h = cx1.astype(f16)
    cx2h = cx2.astype(f16)

    hxT16s, hxT8s = [], []
    for c in range(N_CORES):
        rs = slice(c * BL, (c + 1) * BL)
        A = np.ascontiguousarray(hx[rs]).astype(f16)  # [BL, D]
        hxT16s.append(
            np.ascontiguousarray(A.T.reshape(KT, P, BL).transpose(1, 0, 2))
        )  # [P, KT, BL]
        if FP8_GATES:
            A8 = (np.ascontiguousarray(hx[rs]) * 16.0).astype(E4)
            hxT8s.append(
                np.ascontiguousarray(A8.T.reshape(KT, P, BL).transpose(1, 0, 2))
            )
    return w16, w8, bp, cx1h, cx2h, hxT16s, hxT8s


def kernel(hx, cx1, cx2, tj, dt, W, b, trace=False):
    nc = _get_nc()
    w16, w8, bp, cx1h, cx2h, hxT16s, hxT8s = _pack_host(hx, cx1, cx2, dt, W, b)
    in_maps = []
    for c in range(N_CORES):
        rs = slice(c * BL, (c + 1) * BL)
        im = {
            "hxT16": hxT16s[c],
            "w16": w16,
            "cx1": np.ascontiguousarray(cx1h[rs]),
            "cx2": np.ascontiguousarray(cx2h[rs]),
            "dt": np.ascontiguousarray(dt[rs], dtype=np.float32),
            "bias": bp,
        }
        if FP8_GATES:
            im["hxT8"] = hxT8s[c]
            im["w8"] = w8
        in_maps.append(im)
    res = bass_utils.run_bass_kernel_spmd(
        nc, in_maps, core_ids=list(range(N_CORES)), trace=trace
    )
    out = np.concatenate([r["out"] for r in res.results], axis=1)
    if trace:
        kernel.last_exec_time_ns = res.exec_time_ns
        kernel.last_results = res
    return out
